# revision 30
# baseline (speedup 1.0000x reference)
"""Self-contained Trainium2 Bass kernel for a 1-layer transformer encoder.

Model (fp32 reference):
  x = (emb[input_seq] + pos) * sqrt(D)
  k = x@wk+bk ; q = x@wq+bq ; v = x@wv+bv
  scores[b,i,j] = sum_d k[b,i,d]*q[b,j,d] / sqrt(D)
  attn = softmax(scores, axis=-1) @ v
  r = LN(x + attn) ; ff = gelu(r@w1+b1)@w2+b2 ; out = LN(r + ff)

Sharding: 8 cores; core c handles batch c//2, sequence-half c%2.  Each core
receives its batch's full sequence rolled by -1024*h so its half is local
rows 0..1023 (softmax over keys is permutation-invariant, so one SPMD
program serves both halves).  QKV for the full local sequence is computed
on-core (duplicated across the pair); no collectives.

Precision: matmuls run in float32r (full-rate fp32, ~9 mantissa bits).
The attention-score path is exp-amplified, so it uses a fused matrix
M = wk @ (wq/sqrt(D)).T (host-precomputed; the per-row bias terms cancel
in softmax) and hi/lo-split 3-pass f32r matmuls for near-fp32 accuracy:
  u = x@M   (hi/lo x, hi/lo M);  scores = u.x^T (hi/lo) + 1*t2
where t2 = x @ (wq/sqrt(D) @ bk) carries the bk bias term.
"""

import math

import numpy as np

_B, _S, _D, _DFF, _V = 4, 2048, 512, 2048, 50257
_P = 128
_NCORES = 8
_SQRT_D = math.sqrt(_D)
_EPS = 1e-5

_NT = _S // _P          # 16 sequence tiles
_NI = (_S // 2) // _P   # 8 row tiles per core half
_KC = _D // _P          # 4 contraction chunks over D
_FC = _DFF // _P        # 16 contraction chunks over DFF
_JB = _S // 512         # 4 key blocks of 512

_CACHE = {}


def _pos_table():
    # Mirrors reference pos_embedding in float32.
    pos = np.arange(_S, dtype=np.float32)[:, None]
    i = np.arange(_D, dtype=np.float32)[None, :]
    ang = pos / np.power(np.float32(10000.0), np.float32(2.0) * i / np.float32(_D))
    even = (np.arange(_D) % 2 == 0)[None, :]
    return np.where(even, np.sin(ang), np.cos(ang)).astype(np.float32)


def _round_f32r(a):
    # float32r keeps the top 9 mantissa bits; round-to-nearest on the low 14.
    b = np.ascontiguousarray(a, dtype=np.float32).view(np.uint32)
    b = (b + np.uint32(0x2000)) & np.uint32(0xFFFFC000)
    return b.view(np.float32)


def _split_hi_lo(a64):
    hi = _round_f32r(a64.astype(np.float32))
    lo = _round_f32r((a64 - hi.astype(np.float64)).astype(np.float32))
    return hi, lo


def _build_nc(zero_bk=False, zero_bv=False, zero_b2=False, unit_g=False,
              zero_lb=False):
    import concourse.bass as bass
    import concourse.mybir as mybir
    import concourse.tile as tile
    from concourse import bacc
    from concourse.masks import make_identity

    f32 = mybir.dt.float32
    f32r = mybir.dt.float32r
    i32 = mybir.dt.int32
    AF = mybir.ActivationFunctionType
    OP = mybir.AluOpType
    AX = mybir.AxisListType.X

    nc = bacc.Bacc("TRN2", target_bir_lowering=False, debug=False,
                   num_devices=_NCORES)

    idx_d = nc.dram_tensor("idx", [_P, _NT], i32, kind="ExternalInput")
    # Compact per-core embedding table: host gathers the <=S unique rows this
    # core's batch touches (device still performs the data-dependent gather).
    emb_d = nc.dram_tensor("emb", [_S, _D], f32, kind="ExternalInput")
    pos_d = nc.dram_tensor("pos", [_S, _D], f32, kind="ExternalInput")
    mhi_d = nc.dram_tensor("m_hi", [_D, _D], f32r, kind="ExternalInput")
    mlo_d = nc.dram_tensor("m_lo", [_D, _D], f32r, kind="ExternalInput")
    wv_d = nc.dram_tensor("wv", [_D, _D], f32r, kind="ExternalInput")
    w1_d = nc.dram_tensor("w1", [_D, _DFF], f32r, kind="ExternalInput")
    w2_d = nc.dram_tensor("w2", [_DFF, _D], f32r, kind="ExternalInput")
    c2c_d = nc.dram_tensor("c2c", [_P, _KC], f32r, kind="ExternalInput")
    bvb_d = nc.dram_tensor("bvb", [_P, _D], f32, kind="ExternalInput")
    b1c_d = nc.dram_tensor("b1c", [_P, _FC], f32, kind="ExternalInput")
    b2b_d = nc.dram_tensor("b2b", [_P, _D], f32, kind="ExternalInput")
    gb_d = nc.dram_tensor("gb", [_P, _D], f32, kind="ExternalInput")
    lbb_d = nc.dram_tensor("lbb", [_P, _D], f32, kind="ExternalInput")
    out_d = nc.dram_tensor("out", [_S // 2, _D], f32, kind="ExternalOutput")

    with tile.TileContext(nc) as tc:
        consts = tc.alloc_tile_pool(name="consts", bufs=1)
        id_f = consts.tile([_P, _P], f32, name="id_f")
        make_identity(nc, id_f[:])
        id_r = consts.tile([_P, _P], f32r, name="id_r")
        nc.vector.tensor_copy(out=id_r[:], in_=id_f[:])
        ones_r = consts.tile([1, _P], f32, name="ones_f")
        nc.vector.memset(ones_r[:], 1.0)
        ones_rr = consts.tile([1, _P], f32r, name="ones_rr")
        nc.vector.tensor_copy(out=ones_rr[:], in_=ones_r[:])
        eps_t = consts.tile([_P, 1], f32, name="eps_t")
        nc.vector.memset(eps_t[:], _EPS)
        c2c = bvb = b2b = gb = lbb = None
        if not zero_bk:
            c2c = consts.tile([_P, _KC], f32r, name="c2c")
            nc.scalar.dma_start(out=c2c[:], in_=c2c_d[:, :])
        if not zero_bv:
            bvb = consts.tile([_P, _D], f32, name="bvb")
            nc.scalar.dma_start(out=bvb[:], in_=bvb_d[:, :])
        b1c = consts.tile([_P, _FC], f32, name="b1c")
        nc.scalar.dma_start(out=b1c[:], in_=b1c_d[:, :])
        if not zero_b2:
            b2b = consts.tile([_P, _D], f32, name="b2b")
            nc.scalar.dma_start(out=b2b[:], in_=b2b_d[:, :])
        if not (unit_g and zero_lb):
            gb = consts.tile([_P, _D], f32, name="gb")
            nc.scalar.dma_start(out=gb[:], in_=gb_d[:, :])
            lbb = consts.tile([_P, _D], f32, name="lbb")
            nc.scalar.dma_start(out=lbb[:], in_=lbb_d[:, :])

        xhalf = tc.alloc_tile_pool(name="xhalf", bufs=1)
        x_sb = xhalf.tile([_P, _NI, _D], f32r, name="x_sb")

        acts = tc.alloc_tile_pool(name="acts", bufs=1)
        xT_hi = acts.tile([_P, _KC, _S], f32r, name="xT_hi")
        xT_lo = acts.tile([_P, _KC, _S], f32r, name="xT_lo")
        uT_hi = acts.tile([_P, _KC, _S // 2], f32r, name="uT_hi")
        uT_lo = acts.tile([_P, _KC, _S // 2], f32r, name="uT_lo")
        v_sb = acts.tile([_P, _NT, _D], f32r, name="v_sb")
        t2_sb = None if zero_bk else acts.tile([1, _S], f32r, name="t2_sb")

        # ---------------- Phase 1: embed, x hi/lo, u = x@M, t2, v ----------
        p1 = tc.alloc_tile_pool(name="p1", bufs=1)
        wv_sb = p1.tile([_P, _KC, _D], f32r, name="wv_sb")
        mhi_sb = p1.tile([_P, _KC, _D], f32r, name="mhi_sb")
        mlo_sb = p1.tile([_P, _KC, _D], f32r, name="mlo_sb")
        for c in range(_KC):
            nc.sync.dma_start(out=wv_sb[:, c, :], in_=wv_d[c * _P:(c + 1) * _P, :])
        for c in range(_KC):
            nc.scalar.dma_start(out=mhi_sb[:, c, :], in_=mhi_d[c * _P:(c + 1) * _P, :])

        p1t = tc.alloc_tile_pool(name="p1t", bufs=1)
        idx_sb = p1t.tile([_P, _NT], i32, name="idx_sb")
        nc.sync.dma_start(out=idx_sb[:], in_=idx_d[:, :])
        # Dummy 2-row gather: absorbs the one-time SWDGE descriptor-gen setup
        # (~5us) on the Pool sequencer while idx arrives via the sync queue.
        warm_idx = p1t.tile([2, 1], i32, name="warm_idx")
        nc.gpsimd.memset(warm_idx[:], 0)
        warm_out = p1t.tile([2, _D], f32, name="warm_out")
        nc.gpsimd.indirect_dma_start(
            out=warm_out[:], out_offset=None, in_=emb_d[:, :],
            in_offset=bass.IndirectOffsetOnAxis(ap=warm_idx[:, 0:1], axis=0))

        psp = tc.alloc_tile_pool(name="psp", bufs=1, space="PSUM")

        def emit_ut(ibl):
            rsl = slice(ibl * 512, (ibl + 1) * 512)
            for oc in range(_KC):
                ps_u = psp.tile([_P, 512], f32, name="ps_u", tag="mm", bufs=4)
                first = True
                for (msb, xsb) in ((mhi_sb, xT_hi), (mlo_sb, xT_hi), (mhi_sb, xT_lo)):
                    for c in range(_KC):
                        nc.tensor.matmul(out=ps_u[:],
                                         lhsT=msb[:, c, oc * _P:(oc + 1) * _P],
                                         rhs=xsb[:, c, rsl],
                                         start=first,
                                         stop=(msb is mhi_sb and xsb is xT_lo and c == _KC - 1))
                        first = False
                nc.scalar.activation(out=uT_hi[:, oc, rsl], in_=ps_u[:],
                                     func=AF.Identity, scale=1.0)
                nc.vector.tensor_tensor(out=uT_lo[:, oc, rsl], in0=ps_u[:],
                                        in1=uT_hi[:, oc, rsl], op=OP.subtract)

        def emit_t2(jb):
            ps_m = psp.tile([_P, 512], f32, name="ps_m", tag="mm", bufs=4)
            jsl = slice(jb * 512, (jb + 1) * 512)
            for c in range(_KC):
                nc.tensor.matmul(out=ps_m[0:1, :], lhsT=c2c[:, c:c + 1],
                                 rhs=xT_hi[:, c, jsl],
                                 start=(c == 0), stop=(c == _KC - 1))
            nc.vector.tensor_copy(out=t2_sb[0:1, jsl], in_=ps_m[0:1, :])

        for t in range(_NT):
            if t < _KC:
                # mlo staggered behind pos tiles on the ACT queue; consumed
                # first by emit_ut(0) at t == _NI-1, well after these land.
                nc.scalar.dma_start(out=mlo_sb[:, t, :],
                                    in_=mlo_d[t * _P:(t + 1) * _P, :])
            xg = p1t.tile([_P, _D], f32, name="xg", tag="xg", bufs=3)
            nc.gpsimd.indirect_dma_start(
                out=xg[:], out_offset=None, in_=emb_d[:, :],
                in_offset=bass.IndirectOffsetOnAxis(ap=idx_sb[:, t:t + 1], axis=0))
            pos_t = p1t.tile([_P, _D], f32, name="pos_t", tag="pos_t", bufs=3)
            nc.scalar.dma_start(out=pos_t[:], in_=pos_d[t * _P:(t + 1) * _P, :])
            x_f = p1t.tile([_P, _D], f32, name="x_f", tag="x_f", bufs=3)
            nc.vector.tensor_tensor(out=x_f[:], in0=xg[:], in1=pos_t[:], op=OP.add)
            if t < _NI:
                nc.gpsimd.tensor_copy(out=x_sb[:, t, :], in_=x_f[:])
            ps_x = psp.tile([_P, _KC, _P], f32, name="ps_x", tag="tp", bufs=2)
            for c in range(_KC):
                nc.tensor.transpose(out=ps_x[:, c, :], in_=x_f[:, c * _P:(c + 1) * _P],
                                    identity=id_f[:])
            sl = slice(t * _P, (t + 1) * _P)
            nc.scalar.activation(out=xT_hi[:, :, sl], in_=ps_x[:, :, :],
                                 func=AF.Identity, scale=1.0)
            nc.vector.tensor_tensor(out=xT_lo[:, :, sl], in0=ps_x[:, :, :],
                                    in1=xT_hi[:, :, sl], op=OP.subtract)
            # v for this sequence tile
            ps_v = psp.tile([_P, 512], f32, name="ps_v", tag="mm", bufs=4)
            for c in range(_KC):
                nc.tensor.matmul(out=ps_v[:],
                                 lhsT=xT_hi[:, c, t * _P:(t + 1) * _P],
                                 rhs=wv_sb[:, c, :],
                                 start=(c == 0), stop=(c == _KC - 1))
            if zero_bv:
                nc.scalar.activation(out=v_sb[:, t, :], in_=ps_v[:],
                                     func=AF.Identity, scale=1.0)
            else:
                nc.vector.tensor_tensor(out=v_sb[:, t, :], in0=ps_v[:], in1=bvb[:],
                                        op=OP.add)
            if t == _NI - 1:
                emit_ut(0)
            if t == _NT - 1:
                emit_ut(1)
                if not zero_bk:
                    for jb in range(_JB):
                        emit_t2(jb)

        p1t.release()
        p1.release()

        # ---------------- Phase 2: attention + LN1 ----------------
        rpool = tc.alloc_tile_pool(name="rpool", bufs=1, side="right")
        r_sb = rpool.tile([_P, _NI, _D], f32, name="r_sb")
        preload_w1a = zero_bk and zero_bv and zero_b2 and unit_g and zero_lb
        if preload_w1a:
            w1a = rpool.tile([_P, _KC, _DFF // 2], f32r, name="w1a")
            nc.sync.dma_start(out=w1a[:],
                              in_=w1_d[:, 0:_DFF // 2].rearrange("(c p) n -> p c n", p=_P))

        p2 = tc.alloc_tile_pool(name="p2", bufs=1)

        def emit_ln1(i, ps_a, rinv):
            zt = p2.tile([_P, _D], f32r, name="zt", tag="zt", bufs=1)
            nc.scalar.activation(out=zt[:], in_=ps_a[:], func=AF.Identity,
                                 scale=rinv[:, 0:1])
            z = p2.tile([_P, _D], f32, name="z", tag="z", bufs=1)
            nc.gpsimd.tensor_tensor(out=z[:], in0=zt[:], in1=x_sb[:, i, :], op=OP.add)
            stats = p2.tile([_P, 6], f32, name="stats", tag="stats", bufs=2)
            nc.vector.bn_stats(out=stats[:], in_=z[:])
            mv = p2.tile([_P, 2], f32, name="mv", tag="mv", bufs=2)
            nc.vector.bn_aggr(out=mv[:], in_=stats[:])
            lnv = p2.tile([_P, 1], f32, name="lnv", tag="lnv", bufs=2)
            nc.scalar.activation(out=lnv[:], in_=mv[:, 1:2], func=AF.Ln,
                                 bias=eps_t[:, 0:1], scale=1.0)
            rstd = p2.tile([_P, 1], f32, name="rstd", tag="rstd", bufs=2)
            nc.scalar.activation(out=rstd[:], in_=lnv[:], func=AF.Exp, scale=-0.5)
            if unit_g and zero_lb:
                nc.vector.tensor_scalar(out=r_sb[:, i, :], in0=z[:], scalar1=mv[:, 0:1],
                                        scalar2=rstd[:, 0:1], op0=OP.subtract, op1=OP.mult)
            else:
                t1 = p2.tile([_P, _D], f32, name="t1", tag="t1", bufs=1)
                nc.vector.tensor_scalar(out=t1[:], in0=z[:], scalar1=mv[:, 0:1],
                                        scalar2=rstd[:, 0:1], op0=OP.subtract, op1=OP.mult)
                t2t = p2.tile([_P, _D], f32, name="t2t", tag="t2t", bufs=1)
                nc.gpsimd.tensor_tensor(out=t2t[:], in0=t1[:], in1=gb[:], op=OP.mult)
                nc.gpsimd.tensor_tensor(out=r_sb[:, i, :], in0=t2t[:], in1=lbb[:], op=OP.add)

        pending_ln = None
        for i in range(_NI):
            isl = slice(i * _P, (i + 1) * _P)
            ps_s = []
            m4 = p2.tile([_P, _JB], f32, name="m4", tag="m4", bufs=2)
            for jb in range(_JB):
                ps_sj = psp.tile([_P, 512], f32, name="ps_s", tag="mm", bufs=4)
                ps_s.append(ps_sj)
                jsl = slice(jb * 512, (jb + 1) * 512)
                for (usb, xsb) in ((uT_hi, xT_hi), (uT_hi, xT_lo), (uT_lo, xT_hi)):
                    for c in range(_KC):
                        nc.tensor.matmul(out=ps_sj[:],
                                         lhsT=usb[:, c, isl], rhs=xsb[:, c, jsl],
                                         start=(usb is uT_hi and xsb is xT_hi and c == 0),
                                         stop=(zero_bk and usb is uT_lo and c == _KC - 1))
                if not zero_bk:
                    nc.tensor.matmul(out=ps_sj[:], lhsT=ones_rr[0:1, :],
                                     rhs=t2_sb[0:1, jsl], start=False, stop=True)
                nc.vector.reduce_max(out=m4[:, jb:jb + 1], in_=ps_sj[:], axis=AX)
            mneg = p2.tile([_P, 1], f32, name="mneg", tag="mneg", bufs=2)
            nc.vector.reduce_max(out=mneg[:], in_=m4[:, :], axis=AX, negate=True)
            p_sb = p2.tile([_P, _S], f32r, name="p_sb", tag="p_sb", bufs=1)
            s4 = p2.tile([_P, _JB], f32, name="s4", tag="s4", bufs=2)
            for jb in range(_JB):
                nc.scalar.activation(out=p_sb[:, jb * 512:(jb + 1) * 512],
                                     in_=ps_s[jb][:], func=AF.Exp,
                                     bias=mneg[:, 0:1], scale=1.0,
                                     accum_out=s4[:, jb:jb + 1])
            ssum = p2.tile([_P, 1], f32, name="ssum", tag="ssum", bufs=2)
            nc.vector.reduce_sum(out=ssum[:], in_=s4[:, :], axis=AX)
            rinv = p2.tile([_P, 1], f32, name="rinv", tag="rinv", bufs=2)
            nc.vector.reciprocal(out=rinv[:], in_=ssum[:])

            pT = p2.tile([_P, _NT, _P], f32r, name="pT", tag="pT", bufs=1)
            for g in range(4):
                ps_t = psp.tile([_P, 4, _P], f32r, name="ps_t", tag="tp", bufs=2)
                for q in range(4):
                    jt = 4 * g + q
                    nc.tensor.transpose(out=ps_t[:, q, :],
                                        in_=p_sb[:, jt * _P:(jt + 1) * _P],
                                        identity=id_r[:])
                nc.vector.tensor_copy(out=pT[:, 4 * g:4 * (g + 1), :], in_=ps_t[:, :, :])

            ps_a = psp.tile([_P, _D], f32, name="ps_a", tag="attn", bufs=2)
            for jt in range(_NT):
                nc.tensor.matmul(out=ps_a[:], lhsT=pT[:, jt, :], rhs=v_sb[:, jt, :],
                                 start=(jt == 0), stop=(jt == _NT - 1))
            if pending_ln is not None:
                emit_ln1(*pending_ln)
            pending_ln = (i, ps_a, rinv)
        emit_ln1(*pending_ln)

        p2.release()
        acts.release()
        xhalf.release()

        # ---------------- Phase 3: FFN + LN2 ----------------
        wffn = tc.alloc_tile_pool(name="wffn", bufs=1)
        if not preload_w1a:
            w1a = wffn.tile([_P, _KC, _DFF // 2], f32r, name="w1a")
            nc.sync.dma_start(out=w1a[:],
                              in_=w1_d[:, 0:_DFF // 2].rearrange("(c p) n -> p c n", p=_P))
        w1b = wffn.tile([_P, _KC, _DFF // 2], f32r, name="w1b")
        nc.sync.dma_start(out=w1b[:],
                          in_=w1_d[:, _DFF // 2:].rearrange("(c p) n -> p c n", p=_P))
        w2a = wffn.tile([_P, _FC // 2, _D], f32r, name="w2a")
        nc.sync.dma_start(out=w2a[:],
                          in_=w2_d[0:_DFF // 2, :].rearrange("(c p) n -> p c n", p=_P))
        w2b = wffn.tile([_P, _FC // 2, _D], f32r, name="w2b")
        nc.sync.dma_start(out=w2b[:],
                          in_=w2_d[_DFF // 2:, :].rearrange("(c p) n -> p c n", p=_P))

        p3 = tc.alloc_tile_pool(name="p3", bufs=1)
        rT = p3.tile([_P, _KC, _S // 2], f32r, name="rT")

        def emit_rt(i):
            ps_rt = psp.tile([_P, _KC, _P], f32, name="ps_rt", tag="tp", bufs=2)
            for c in range(_KC):
                nc.tensor.transpose(out=ps_rt[:, c, :],
                                    in_=r_sb[:, i, c * _P:(c + 1) * _P],
                                    identity=id_f[:])
            nc.vector.tensor_copy(out=rT[:, :, i * _P:(i + 1) * _P], in_=ps_rt[:, :, :])

        for i in range(4):
            emit_rt(i)
        gT0 = p3.tile([_P, _FC, 512], f32r, name="gT0")
        gT1 = p3.tile([_P, _FC, 512], f32r, name="gT1")
        for ib, gT in ((0, gT0), (1, gT1)):
            if ib == 1:
                for i in range(4, _NI):
                    emit_rt(i)
            for fc in range(_FC):
                ps_h = psp.tile([_P, 512], f32, name="ps_h", tag="mm", bufs=4)
                w1h = w1a if fc < _FC // 2 else w1b
                fcl = fc if fc < _FC // 2 else fc - _FC // 2
                for c in range(_KC):
                    nc.tensor.matmul(out=ps_h[:],
                                     lhsT=w1h[:, c, fcl * _P:(fcl + 1) * _P],
                                     rhs=rT[:, c, ib * 512:(ib + 1) * 512],
                                     start=(c == 0), stop=(c == _KC - 1))
                nc.scalar.activation(out=gT[:, fc, :], in_=ps_h[:], func=AF.Gelu,
                                     bias=b1c[:, fc:fc + 1], scale=1.0)
        for i in range(_NI):
            ib, il = divmod(i, 4)
            gT = gT0 if ib == 0 else gT1
            ps_o = psp.tile([_P, _D], f32, name="ps_o", tag="attn", bufs=2)
            for fc in range(_FC):
                w2h = w2a if fc < _FC // 2 else w2b
                fcl = fc if fc < _FC // 2 else fc - _FC // 2
                nc.tensor.matmul(out=ps_o[:],
                                 lhsT=gT[:, fc, il * _P:(il + 1) * _P],
                                 rhs=w2h[:, fcl, :],
                                 start=(fc == 0), stop=(fc == _FC - 1))
            t3 = p3.tile([_P, _D], f32, name="t3", tag="t3", bufs=2)
            nc.vector.tensor_tensor(out=t3[:], in0=ps_o[:], in1=r_sb[:, i, :],
                                    op=OP.add)
            if zero_b2:
                z2 = t3
            else:
                z2 = p3.tile([_P, _D], f32, name="z2", tag="z2", bufs=2)
                nc.gpsimd.tensor_tensor(out=z2[:], in0=t3[:], in1=b2b[:], op=OP.add)
            stats2 = p3.tile([_P, 6], f32, name="stats2", tag="stats2", bufs=2)
            nc.vector.bn_stats(out=stats2[:], in_=z2[:])
            mv2 = p3.tile([_P, 2], f32, name="mv2", tag="mv2", bufs=2)
            nc.vector.bn_aggr(out=mv2[:], in_=stats2[:])
            std2 = p3.tile([_P, 1], f32, name="std2", tag="std2", bufs=2)
            nc.scalar.activation(out=std2[:], in_=mv2[:, 1:2], func=AF.Sqrt,
                                 bias=eps_t[:, 0:1], scale=1.0)
            rstd2 = p3.tile([_P, 1], f32, name="rstd2", tag="rstd2", bufs=2)
            nc.vector.reciprocal(out=rstd2[:], in_=std2[:])
            out_t = p3.tile([_P, _D], f32, name="out_t", tag="out_t", bufs=3)
            if unit_g and zero_lb:
                nc.vector.tensor_scalar(out=out_t[:], in0=z2[:], scalar1=mv2[:, 0:1],
                                        scalar2=rstd2[:, 0:1],
                                        op0=OP.subtract, op1=OP.mult)
            else:
                t4 = p3.tile([_P, _D], f32, name="t4", tag="t4", bufs=2)
                nc.vector.tensor_scalar(out=t4[:], in0=z2[:], scalar1=mv2[:, 0:1],
                                        scalar2=rstd2[:, 0:1],
                                        op0=OP.subtract, op1=OP.mult)
                t5 = p3.tile([_P, _D], f32, name="t5", tag="t5", bufs=2)
                nc.gpsimd.tensor_tensor(out=t5[:], in0=t4[:], in1=gb[:], op=OP.mult)
                nc.gpsimd.tensor_tensor(out=out_t[:], in0=t5[:], in1=lbb[:], op=OP.add)
            nc.sync.dma_start(out=out_d[i * _P:(i + 1) * _P, :], in_=out_t[:])

        psp.release()
        p3.release()
        wffn.release()
        rpool.release()
        consts.release()

    nc.compile()
    return nc


def _get_nc(flags=(False, False, False, False, False)):
    if flags not in _CACHE:
        _CACHE[flags] = _build_nc(*flags)
    return _CACHE[flags]


def _make_in_maps(inp):
    f32 = np.float32
    emb_full = np.asarray(inp["emb"])
    pos_s = _pos_table() * f32(_SQRT_D)

    wk64 = np.asarray(inp["wk"], np.float64)
    wqp64 = np.asarray(inp["wq"], np.float64) / _SQRT_D
    m64 = wk64 @ wqp64.T
    m_hi, m_lo = _split_hi_lo(m64)
    c2 = (wqp64 @ np.asarray(inp["bk"], np.float64)).astype(f32)

    def col(bias, nchunk):
        return np.ascontiguousarray(np.asarray(bias, f32).reshape(nchunk, _P).T)

    def bcast(bias):
        return np.ascontiguousarray(np.broadcast_to(np.asarray(bias, f32), (_P, _D)))

    shared = {
        "m_hi": np.ascontiguousarray(m_hi),
        "m_lo": np.ascontiguousarray(m_lo),
        "wv": np.ascontiguousarray(inp["wv"], dtype=f32),
        "w1": np.ascontiguousarray(inp["w1"], dtype=f32),
        "w2": np.ascontiguousarray(inp["w2"], dtype=f32),
        "c2c": col(_round_f32r(c2), _KC),
        "bvb": bcast(inp["bv"]),
        "b1c": col(inp["b1"], _FC),
        "b2b": bcast(inp["b2"]),
        "gb": bcast(inp["ln_g"]),
        "lbb": bcast(inp["ln_b"]),
    }
    in_maps = []
    for core in range(_NCORES):
        b, h = divmod(core, 2)
        seq = np.asarray(inp["input_seq"][b]).astype(np.int64)
        seq = np.roll(seq, -1024 * h)
        uniq, inv = np.unique(seq, return_inverse=True)
        emb_c = np.zeros((_S, _D), f32)
        emb_c[:len(uniq)] = emb_full[uniq]
        emb_c[:len(uniq)] *= f32(_SQRT_D)
        m = dict(shared)
        m["emb"] = emb_c
        m["idx"] = np.ascontiguousarray(inv.astype(np.int32).reshape(_NT, _P).T)
        m["pos"] = np.ascontiguousarray(np.roll(pos_s, -1024 * h, axis=0))
        in_maps.append(m)
    return in_maps


def kernel(**inputs):
    from concourse.bass_utils import run_bass_kernel_spmd

    inp = {k: np.asarray(v) for k, v in inputs.items()}
    in_maps = _make_in_maps(inp)
    flags = (bool(np.all(np.asarray(inp["bk"]) == 0)),
             bool(np.all(np.asarray(inp["bv"]) == 0)),
             bool(np.all(np.asarray(inp["b2"]) == 0)),
             bool(np.all(np.asarray(inp["ln_g"]) == 1)),
             bool(np.all(np.asarray(inp["ln_b"]) == 0)))
    nc = _get_nc(flags)
    res = run_bass_kernel_spmd(nc, in_maps, core_ids=list(range(_NCORES)))
    out = np.empty((_B, _S, _D), np.float32)
    for core in range(_NCORES):
        b, h = divmod(core, 2)
        out[b, h * 1024:(h + 1) * 1024, :] = res.results[core]["out"]
    return out


if __name__ == "__main__":
    import sys
    if "--build" in sys.argv:
        import tempfile
        from concourse.bass_utils import compile_bass_kernel
        nc = _build_nc(True, True, True, True, True)
        d = tempfile.mkdtemp(prefix="enc_build_")
        print("compiling into", d)
        print("NEFF:", compile_bass_kernel(nc, d))
